# revision 16
# baseline (speedup 1.0000x reference)
"""Multi-head attention (b=2, n=2048, e=1024, h=16, d=64) on 8 trn2 NeuronCores.

Sharding: data-parallel over batch (2) x tensor-parallel over heads (16 -> 4
groups of 4). Core c handles batch c//4, heads 4*(c%4) .. 4*(c%4)+3.
Each core computes the qkv projection for its heads, full attention, and a
row-parallel slice of the output projection; the host sums the 4 partial
projections per batch and adds bproj.

On-chip layout is fully "transposed" (features on partitions) so softmax runs
along the free axis and no on-chip transposes are needed:
  qT,kT [dcol, tok]   (dcol = head-major 4*64=256, two 128-partition chunks)
  v     [tok, dcol]   (natural), stored per l-tile with a ones column appended
  energyT[l, q] = kT.T @ qT   per head (K=64, head pairs at partitions 0/64)
  expT = exp(energyT/32)  (ScalarE, bf16 out)
  att@v: psum[65, q] += v_ext[l,65].T @ expT[l, q]  -- row 64 accumulates the
  softmax denominator for free (ones column).
  normalize with reciprocal_approx_fast + DRAM-round-trip partition broadcast
  proj: out_partial[q, e] = outT.T @ Wproj_slice  (row-parallel, host reduces)

Emission order interleaves phases so ScalarE (exp) keeps busy while the PE
does the second head-pair's projections, and alternates h/h+1 attention units
so energy matmuls land on disjoint PE row groups (base partitions 0/64).
"""

import numpy as np
import ml_dtypes

import concourse.bass as bass
import concourse.tile as tile
from concourse import bacc, mybir
from concourse import bass_utils

B, N, E, H, D = 2, 2048, 1024, 16, 64
NCORES = 8
HPC = H // 4  # heads per core = 4
DC = HPC * D  # dcols per core = 256
EC = E // 128  # 8 e-chunks
NT = N // 128  # 16 token tiles
QC = N // 512  # 4 q-chunks of 512
F32 = mybir.dt.float32
BF16 = mybir.dt.bfloat16
BF = ml_dtypes.bfloat16

_CACHE = {}


def build_nc(debug_outs=False, reps=None):
    nc = bacc.Bacc("TRN2", target_bir_lowering=False, debug=False, num_devices=NCORES)

    xT_d = nc.dram_tensor("xT", [E, N], BF16, kind="ExternalInput")
    wq_d = nc.dram_tensor("wq", [E, DC], BF16, kind="ExternalInput")
    wk_d = nc.dram_tensor("wk", [E, DC], BF16, kind="ExternalInput")
    wv_d = nc.dram_tensor("wv", [E, DC], BF16, kind="ExternalInput")
    wp_d = nc.dram_tensor("wp", [DC, E], BF16, kind="ExternalInput")
    bqT_d = nc.dram_tensor("bqT", [DC, 1], F32, kind="ExternalInput")
    bkT_d = nc.dram_tensor("bkT", [DC, 1], F32, kind="ExternalInput")
    bvb_d = nc.dram_tensor("bvb", [128, DC], F32, kind="ExternalInput")
    out_d = nc.dram_tensor("out", [N, E], F32, kind="ExternalOutput")

    with tile.TileContext(nc) as tc:
        with (
            tc.tile_pool(name="const", bufs=1) as const,
            tc.tile_pool(name="ps", bufs=2, space="PSUM") as ps_pool,
            tc.tile_pool(name="stg", bufs=2, space="PSUM") as stg_pool,
            tc.tile_pool(name="av", bufs=2, space="PSUM") as av_pool,
            tc.tile_pool(name="ex", bufs=3) as ex_pool,
            tc.tile_pool(name="nrm", bufs=3) as nrm_pool,
            tc.tile_pool(name="dscr", bufs=3, space="DRAM") as dscr_pool,
            tc.tile_pool(name="outst", bufs=4) as outst_pool,
        ):
            def emit_body():
                # ---- persistent SBUF tensors ----
                xT_sb = const.tile([128, EC, N], BF16)
                wq_sb = const.tile([128, EC, DC], BF16)
                wk_sb = const.tile([128, EC, DC], BF16)
                wv_sb = const.tile([128, EC, DC], BF16)
                wp_sb = const.tile([128, 2, E], BF16)
                bqT_sb = const.tile([128, 2], F32)
                bkT_sb = const.tile([128, 2], F32)
                bvb_sb = const.tile([128, DC], F32)
                qT_sb = const.tile([128, 2, N], BF16)
                kT_sb = const.tile([128, 2, N], BF16)
                # per l-tile, per head: 64 dims + ones col + pad -> lhsT [128, 65]
                v_sb = const.tile([128, NT, HPC, 66], BF16)
                outT_sb = const.tile([128, 2, N], BF16)

                # ---- input DMAs (wv early: v is needed by every head) ----
                for ec in range(EC):
                    nc.sync.dma_start(out=xT_sb[:, ec, :], in_=xT_d.ap()[ec * 128:(ec + 1) * 128, :])
                    nc.sync.dma_start(out=wv_sb[:, ec, :], in_=wv_d.ap()[ec * 128:(ec + 1) * 128, :])
                    nc.sync.dma_start(out=wq_sb[:, ec, :], in_=wq_d.ap()[ec * 128:(ec + 1) * 128, :])
                    nc.sync.dma_start(out=wk_sb[:, ec, :], in_=wk_d.ap()[ec * 128:(ec + 1) * 128, :])
                for hc in range(2):
                    nc.sync.dma_start(out=wp_sb[:, hc, :], in_=wp_d.ap()[hc * 128:(hc + 1) * 128, :])
                    nc.sync.dma_start(out=bqT_sb[:, hc:hc + 1], in_=bqT_d.ap()[hc * 128:(hc + 1) * 128, :])
                    nc.sync.dma_start(out=bkT_sb[:, hc:hc + 1], in_=bkT_d.ap()[hc * 128:(hc + 1) * 128, :])
                nc.sync.dma_start(out=bvb_sb[:], in_=bvb_d.ap())

                nc.vector.memset(v_sb[:, :, :, 64:65], 1.0)

                inv_scale = 1.0 / float(np.sqrt(np.float32(E)))

                # ---- emit qkv projection for one dcol chunk (2 heads) ----
                def emit_qk(m):
                    for w_sb, b_sb, dst in ((wq_sb, bqT_sb, qT_sb), (wk_sb, bkT_sb, kT_sb)):
                        for t in range(QC):
                            pq = ps_pool.tile([128, 512], F32, tag="ps")
                            for ec in range(EC):
                                nc.tensor.matmul(
                                    pq[:],
                                    lhsT=w_sb[:, ec, m * 128:(m + 1) * 128],
                                    rhs=xT_sb[:, ec, t * 512:(t + 1) * 512],
                                    start=(ec == 0), stop=(ec == EC - 1),
                                )
                            nc.vector.tensor_scalar_add(
                                out=dst[:, m, t * 512:(t + 1) * 512],
                                in0=pq[:], scalar1=b_sb[:, m:m + 1],
                            )

                def emit_v():
                    for lt in range(NT):
                        pv = ps_pool.tile([128, DC], F32, tag="ps")
                        for ec in range(EC):
                            nc.tensor.matmul(
                                pv[:],
                                lhsT=xT_sb[:, ec, lt * 128:(lt + 1) * 128],
                                rhs=wv_sb[:, ec, :],
                                start=(ec == 0), stop=(ec == EC - 1),
                            )
                        nc.vector.tensor_add(
                            out=v_sb[:, lt, :, 0:64],
                            in0=pv[:].rearrange("p (h d) -> p h d", h=HPC),
                            in1=bvb_sb[:].rearrange("p (h d) -> p h d", h=HPC),
                        )

                # ---- one attention unit: head h, q-chunk qc ----
                def emit_att_unit(h, qc):
                    hc, hp = h // 2, (h % 2) * 64
                    av = av_pool.tile([65, 512], F32, tag="av")
                    for r in range(NT // 2):  # staging rounds: 2 l-tiles each
                        stg = stg_pool.tile([128, 2, 512], F32, tag="stg")
                        for j in range(2):
                            lt = r * 2 + j
                            nc.tensor.matmul(
                                stg[:, j, :],
                                lhsT=kT_sb[hp:hp + 64, hc, lt * 128:(lt + 1) * 128],
                                rhs=qT_sb[hp:hp + 64, hc, qc * 512:(qc + 1) * 512],
                                start=True, stop=True,
                            )
                        ex = ex_pool.tile([128, 2, 512], BF16, tag="ex")
                        nc.scalar.activation(
                            out=ex[:], in_=stg[:],
                            func=mybir.ActivationFunctionType.Exp,
                            scale=inv_scale,
                        )
                        for j in range(2):
                            lt = r * 2 + j
                            nc.tensor.matmul(
                                av[:],
                                lhsT=v_sb[:, lt, h, 0:65],
                                rhs=ex[:, j, :],
                                start=(lt == 0), stop=(lt == NT - 1),
                            )
                    # normalization (custom DVE ops need partition-0 input)
                    den_sb = nrm_pool.tile([1, 512], F32, tag="den")
                    nc.vector.tensor_copy(out=den_sb[:], in_=av[64:65, :])
                    recip = nrm_pool.tile([1, 512], F32, tag="recip")
                    nc.vector.reciprocal_approx_fast(out=recip[:], in_=den_sb[:])
                    dscr = dscr_pool.tile([1, 512], F32, tag="dscr")
                    nc.sync.dma_start(out=dscr[:], in_=recip[:])
                    rbc = nrm_pool.tile([64, 512], F32, tag="rbc")
                    d_ap = dscr[:]
                    bcast = bass.AP(tensor=d_ap.tensor, offset=d_ap.offset,
                                    ap=[[0, 64]] + list(d_ap.ap[1:]))
                    nc.sync.dma_start(out=rbc[:], in_=bcast)
                    nc.vector.tensor_mul(
                        out=outT_sb[hp:hp + 64, hc, qc * 512:(qc + 1) * 512],
                        in0=av[0:64, :],
                        in1=rbc[:],
                    )

                # ---- phase interleaving ----
                emit_v()
                emit_qk(0)
                for qc in range(QC):       # heads 0,1 alternate (row groups 0/64)
                    emit_att_unit(0, qc)
                    emit_att_unit(1, qc)
                emit_qk(1)
                for qc in range(QC):
                    emit_att_unit(2, qc)
                    emit_att_unit(3, qc)

                # ---- output projection (evac split across DVE and ScalarE) ----
                for qt in range(NT):
                    for en in range(2):
                        po = ps_pool.tile([128, 512], F32, tag="ps")
                        for hc in range(2):
                            nc.tensor.matmul(
                                po[:],
                                lhsT=outT_sb[:, hc, qt * 128:(qt + 1) * 128],
                                rhs=wp_sb[:, hc, en * 512:(en + 1) * 512],
                                start=(hc == 0), stop=(hc == 1),
                            )
                        ot = outst_pool.tile([128, 512], F32, tag="ot")
                        if en == 0:
                            nc.vector.tensor_copy(out=ot[:], in_=po[:])
                        else:
                            nc.scalar.copy(out=ot[:], in_=po[:])
                        nc.sync.dma_start(
                            out=out_d.ap()[qt * 128:(qt + 1) * 128, en * 512:(en + 1) * 512],
                            in_=ot[:],
                        )

            if reps is None:
                emit_body()
            else:
                with tc.For_i(0, reps, 1, hint_engines=(
                        mybir.EngineType.PE, mybir.EngineType.Activation,
                        mybir.EngineType.DVE, mybir.EngineType.SP)):
                    emit_body()

    nc.compile()
    return nc


def make_in_maps(x, Wqkv, bqkv, Wproj):
    W4 = np.ascontiguousarray(Wqkv.reshape(E, H, D, 3))
    b4 = np.ascontiguousarray(bqkv.reshape(H, D, 3))
    in_maps = []
    for c in range(NCORES):
        bi, hg = c // 4, c % 4
        hs = slice(hg * HPC, (hg + 1) * HPC)
        in_maps.append({
            "xT": np.ascontiguousarray(x[bi].T).astype(BF),
            "wq": np.ascontiguousarray(W4[:, hs, :, 0].reshape(E, DC)).astype(BF),
            "wk": np.ascontiguousarray(W4[:, hs, :, 1].reshape(E, DC)).astype(BF),
            "wv": np.ascontiguousarray(W4[:, hs, :, 2].reshape(E, DC)).astype(BF),
            "wp": np.ascontiguousarray(Wproj[hg * DC:(hg + 1) * DC, :]).astype(BF),
            "bqT": np.ascontiguousarray(b4[hs, :, 0].reshape(DC, 1)).astype(np.float32),
            "bkT": np.ascontiguousarray(b4[hs, :, 1].reshape(DC, 1)).astype(np.float32),
            "bvb": np.ascontiguousarray(np.tile(b4[hs, :, 2].reshape(1, DC), (128, 1))).astype(np.float32),
        })
    return in_maps


def run(inputs, trace=False, **kw):
    if "nc" not in _CACHE:
        _CACHE["nc"] = build_nc()
    nc = _CACHE["nc"]
    in_maps = make_in_maps(inputs["x"], inputs["Wqkv"], inputs["bqkv"], inputs["Wproj"])
    res = bass_utils.run_bass_kernel_spmd(nc, in_maps, core_ids=list(range(NCORES)), trace=trace, **kw)
    out = np.zeros((B, N, E), np.float32)
    for c in range(NCORES):
        out[c // 4] += res.results[c]["out"]
    out += inputs["bproj"].astype(np.float32)[None, None, :]
    return out, res


def kernel(**inputs):
    inputs = {k: np.asarray(v) for k, v in inputs.items()}
    out, _ = run(inputs)
    return out.astype(np.float32)


# revision 41
# speedup vs baseline: 12212.8655x; 12212.8655x over previous
"""Multi-head attention (b=2, n=2048, e=1024, h=16, d=64) on 8 trn2 NeuronCores.

Sharding: data-parallel over batch (2) x tensor-parallel over heads (16 -> 4
groups of 4). Core c handles batch c//4, heads 4*(c%4) .. 4*(c%4)+3.
Each core computes the qkv projection for its heads, full attention, and a
row-parallel slice of the output projection; the host sums the 4 partial
projections per batch and adds bproj.

On-chip layout is fully "transposed" (features on partitions) so softmax runs
along the free axis and no on-chip transposes are needed:
  qT,kT [dcol, tok]   (dcol = head-major 4*64=256, two 128-partition chunks)
  v     [tok, dcol]   (natural), stored per l-tile with a ones column appended
  energyT[l, q] = kT.T @ qT   per head (K=64, head pairs at partitions 0/64)
  expT = exp(energyT/32)  (ScalarE, bf16 out)
  att@v: psum[65, q] += v_ext[l,65].T @ expT[l, q]  -- row 64 accumulates the
  softmax denominator for free (ones column).
  normalize with reciprocal_approx_fast + DRAM-round-trip partition broadcast
  proj: out_partial[q, e] = outT.T @ Wproj_slice  (row-parallel, host reduces)

Schedule notes:
 - head pairs are processed with round-level interleave so consecutive energy
   matmuls target disjoint PE row groups (base partitions 0/64) and overlap in
   the systolic array
 - the v projection is emitted just-in-time inside the first attention unit
   so ScalarE starts exp work ~14us earlier
 - attention accumulators are evacuated from PSUM to SBUF immediately so the
   2 accumulator banks recycle at once; normalization runs from SBUF off the
   critical path
"""

import numpy as np
import ml_dtypes

import concourse.bass as bass
import concourse.tile as tile
from concourse import bacc, mybir
from concourse import bass_utils

B, N, E, H, D = 2, 2048, 1024, 16, 64
NCORES = 8
HPC = H // 4  # heads per core = 4
DC = HPC * D  # dcols per core = 256
EC = E // 128  # 8 e-chunks
NT = N // 128  # 16 token tiles
QC = N // 512  # 4 q-chunks of 512
F32 = mybir.dt.float32
BF16 = mybir.dt.bfloat16
BF = ml_dtypes.bfloat16

_CACHE = {}


def build_nc(debug_outs=False, reps=None, tiny_out=False):
    nc = bacc.Bacc("TRN2", target_bir_lowering=False, debug=False, num_devices=NCORES)

    xT_d = nc.dram_tensor("xT", [E, N], BF16, kind="ExternalInput")
    wq_d = nc.dram_tensor("wq", [E, DC], BF16, kind="ExternalInput")
    wk_d = nc.dram_tensor("wk", [E, DC], BF16, kind="ExternalInput")
    wv_d = nc.dram_tensor("wv", [E, DC], BF16, kind="ExternalInput")
    wp_d = nc.dram_tensor("wp", [DC, E], BF16, kind="ExternalInput")
    bqT_d = nc.dram_tensor("bqT", [DC, 1], F32, kind="ExternalInput")
    bkT_d = nc.dram_tensor("bkT", [DC, 1], F32, kind="ExternalInput")
    bvb_d = nc.dram_tensor("bvb", [128, DC], F32, kind="ExternalInput")
    out_rows = 512 if tiny_out else N
    out_d = nc.dram_tensor("out", [out_rows, E], F32, kind="ExternalOutput")

    with tile.TileContext(nc) as tc:
        with (
            tc.tile_pool(name="const", bufs=1) as const,
            tc.tile_pool(name="ps", bufs=2, space="PSUM") as ps_pool,
            tc.tile_pool(name="stg", bufs=2, space="PSUM") as stg_pool,
            tc.tile_pool(name="av", bufs=2, space="PSUM") as av_pool,
            tc.tile_pool(name="ex", bufs=6) as ex_pool,
            tc.tile_pool(name="nrm", bufs=3) as nrm_pool,
            tc.tile_pool(name="dscr", bufs=3, space="DRAM") as dscr_pool,
            tc.tile_pool(name="outst", bufs=4) as outst_pool,
        ):
            def emit_body():
                # ---- persistent SBUF tensors ----
                xT_sb = const.tile([128, EC, N], BF16)
                wq_sb = const.tile([128, EC, DC], BF16)
                wk_sb = const.tile([128, EC, DC], BF16)
                wv_sb = const.tile([128, EC, DC], BF16)
                wp_sb = const.tile([128, 2, E], BF16)
                bqT_sb = const.tile([128, 2], F32)
                bkT_sb = const.tile([128, 2], F32)
                bvb_sb = const.tile([128, DC], F32)
                qT_sb = const.tile([128, 2, N], BF16)
                kT_sb = const.tile([128, 2, N], BF16)
                # per l-tile, per head: 64 dims + ones col + pad -> lhsT [128, 65]
                v_sb = const.tile([128, NT, HPC, 66], BF16)
                outT_sb = const.tile([128, 2, N], BF16)

                # ---- input DMAs spread over queue engines ----
                qeng = [nc.sync, nc.scalar]
                qi = [0]

                def dma(out, in_):
                    qeng[qi[0] % len(qeng)].dma_start(out=out, in_=in_)
                    qi[0] += 1

                for ec in range(EC):
                    dma(xT_sb[:, ec, :], xT_d.ap()[ec * 128:(ec + 1) * 128, :])
                    dma(wk_sb[:, ec, :], wk_d.ap()[ec * 128:(ec + 1) * 128, :])
                    dma(wq_sb[:, ec, :], wq_d.ap()[ec * 128:(ec + 1) * 128, :])
                    dma(wv_sb[:, ec, :], wv_d.ap()[ec * 128:(ec + 1) * 128, :])
                for hc in range(2):
                    dma(wp_sb[:, hc, :], wp_d.ap()[hc * 128:(hc + 1) * 128, :])
                    dma(bqT_sb[:, hc:hc + 1], bqT_d.ap()[hc * 128:(hc + 1) * 128, :])
                    dma(bkT_sb[:, hc:hc + 1], bkT_d.ap()[hc * 128:(hc + 1) * 128, :])
                dma(bvb_sb[:], bvb_d.ap())

                nc.vector.memset(v_sb[:, :, :, 64:65], 1.0)

                inv_scale = 1.0 / float(np.sqrt(np.float32(E)))

                QK = ((wk_sb, bkT_sb, kT_sb), (wq_sb, bqT_sb, qT_sb))

                def emit_qk_group(which, m, t):
                    w_sb, b_sb, dst = QK[which]
                    pq = ps_pool.tile([128, 512], F32, tag="ps")
                    for ec in range(EC):
                        nc.tensor.matmul(
                            pq[:],
                            lhsT=w_sb[:, ec, m * 128:(m + 1) * 128],
                            rhs=xT_sb[:, ec, t * 512:(t + 1) * 512],
                            start=(ec == 0), stop=(ec == EC - 1),
                        )
                    nc.vector.tensor_scalar_add(
                        out=dst[:, m, t * 512:(t + 1) * 512],
                        in0=pq[:], scalar1=b_sb[:, m:m + 1],
                    )

                def emit_qk(m):
                    for which in range(2):
                        for t in range(QC):
                            emit_qk_group(which, m, t)

                def emit_v_group(lt):
                    pv = ps_pool.tile([128, DC], F32, tag="ps")
                    for ec in range(EC):
                        nc.tensor.matmul(
                            pv[:],
                            lhsT=xT_sb[:, ec, lt * 128:(lt + 1) * 128],
                            rhs=wv_sb[:, ec, :],
                            start=(ec == 0), stop=(ec == EC - 1),
                        )
                    nc.vector.tensor_add(
                        out=v_sb[:, lt, :, 0:64],
                        in0=pv[:].rearrange("p (h d) -> p h d", h=HPC),
                        in1=bvb_sb[:].rearrange("p (h d) -> p h d", h=HPC),
                    )

                # ---- attention unit: one head, one q-chunk ----
                def emit_att_unit(h, qc, emit_v=False):
                    hc, hp = h // 2, (h % 2) * 64
                    av = av_pool.tile([65, 512], F32, tag="av")
                    for r in range(NT // 2):
                        if emit_v:
                            emit_v_group(2 * r)
                            emit_v_group(2 * r + 1)
                        stg = stg_pool.tile([128, 2, 512], F32, tag="stg")
                        for j in range(2):
                            lt = r * 2 + j
                            nc.tensor.matmul(
                                stg[:, j, :],
                                lhsT=kT_sb[hp:hp + 64, hc, lt * 128:(lt + 1) * 128],
                                rhs=qT_sb[hp:hp + 64, hc, qc * 512:(qc + 1) * 512],
                                start=True, stop=True,
                            )
                        ex = ex_pool.tile([128, 2, 512], BF16, tag="ex")
                        nc.scalar.activation(
                            out=ex[:], in_=stg[:],
                            func=mybir.ActivationFunctionType.Exp,
                            scale=inv_scale,
                        )
                        for j in range(2):
                            lt = r * 2 + j
                            nc.tensor.matmul(
                                av[:],
                                lhsT=v_sb[:, lt, h, 0:65],
                                rhs=ex[:, j, :],
                                start=(lt == 0), stop=(lt == NT - 1),
                            )
                    av_sb = nrm_pool.tile([65, 512], F32, tag="avsb", bufs=4)
                    nc.vector.tensor_copy(out=av_sb[:], in_=av[:])
                    den_sb = nrm_pool.tile([1, 512], F32, tag="den")
                    nc.vector.tensor_copy(out=den_sb[:], in_=av_sb[64:65, :])
                    recip = nrm_pool.tile([1, 512], F32, tag="recip")
                    nc.vector.reciprocal_approx_fast(out=recip[:], in_=den_sb[:])
                    dscr = dscr_pool.tile([1, 512], F32, tag="dscr")
                    nc.sync.dma_start(out=dscr[:], in_=recip[:])
                    rbc = nrm_pool.tile([64, 512], F32, tag="rbc")
                    d_ap = dscr[:]
                    bcast = bass.AP(tensor=d_ap.tensor, offset=d_ap.offset,
                                    ap=[[0, 64]] + list(d_ap.ap[1:]))
                    nc.sync.dma_start(out=rbc[:], in_=bcast)
                    nc.vector.tensor_mul(
                        out=outT_sb[hp:hp + 64, hc, qc * 512:(qc + 1) * 512],
                        in0=av_sb[0:64, :],
                        in1=rbc[:],
                    )

                # one output-projection column block (q-tile qt, 1024 wide)
                def emit_proj(qt):
                    for en in range(2):
                        po = ps_pool.tile([128, 512], F32, tag="ps")
                        for hc in range(2):
                            nc.tensor.matmul(
                                po[:],
                                lhsT=outT_sb[:, hc, qt * 128:(qt + 1) * 128],
                                rhs=wp_sb[:, hc, en * 512:(en + 1) * 512],
                                start=(hc == 0), stop=(hc == 1),
                            )
                        ot = outst_pool.tile([128, 512], F32, tag="ot")
                        nc.vector.tensor_copy(out=ot[:], in_=po[:])
                        oq = (qt % 4) if tiny_out else qt
                        (nc.sync if en == 0 else nc.scalar).dma_start(
                            out=out_d.ap()[oq * 128:(oq + 1) * 128, en * 512:(en + 1) * 512],
                            in_=ot[:],
                        )

                # ---- phase interleaving ----
                emit_qk(0)
                emit_att_unit(0, 0, emit_v=True)
                emit_att_unit(1, 0)
                for qc in range(1, QC):
                    emit_att_unit(0, qc)
                    emit_att_unit(1, qc)
                emit_qk(1)
                for qc in range(QC):
                    emit_att_unit(2, qc)
                    emit_att_unit(3, qc)
                for qt in range(NT):
                    emit_proj(qt)

            if reps is None:
                emit_body()
            else:
                with tc.For_i(0, reps, 1, hint_engines=(
                        mybir.EngineType.PE, mybir.EngineType.Activation,
                        mybir.EngineType.DVE, mybir.EngineType.SP)):
                    emit_body()

    nc.compile()
    return nc


def make_in_maps(x, Wqkv, bqkv, Wproj):
    W4 = np.ascontiguousarray(Wqkv.reshape(E, H, D, 3))
    b4 = np.ascontiguousarray(bqkv.reshape(H, D, 3))
    in_maps = []
    for c in range(NCORES):
        bi, hg = c // 4, c % 4
        hs = slice(hg * HPC, (hg + 1) * HPC)
        in_maps.append({
            "xT": np.ascontiguousarray(x[bi].T).astype(BF),
            "wq": np.ascontiguousarray(W4[:, hs, :, 0].reshape(E, DC)).astype(BF),
            "wk": np.ascontiguousarray(W4[:, hs, :, 1].reshape(E, DC)).astype(BF),
            "wv": np.ascontiguousarray(W4[:, hs, :, 2].reshape(E, DC)).astype(BF),
            "wp": np.ascontiguousarray(Wproj[hg * DC:(hg + 1) * DC, :]).astype(BF),
            "bqT": np.ascontiguousarray(b4[hs, :, 0].reshape(DC, 1)).astype(np.float32),
            "bkT": np.ascontiguousarray(b4[hs, :, 1].reshape(DC, 1)).astype(np.float32),
            "bvb": np.ascontiguousarray(np.tile(b4[hs, :, 2].reshape(1, DC), (128, 1))).astype(np.float32),
        })
    return in_maps


def run(inputs, trace=False, **kw):
    if "nc" not in _CACHE:
        _CACHE["nc"] = build_nc()
    nc = _CACHE["nc"]
    in_maps = make_in_maps(inputs["x"], inputs["Wqkv"], inputs["bqkv"], inputs["Wproj"])
    res = bass_utils.run_bass_kernel_spmd(nc, in_maps, core_ids=list(range(NCORES)), trace=trace, **kw)
    out = np.zeros((B, N, E), np.float32)
    for c in range(NCORES):
        out[c // 4] += res.results[c]["out"].astype(np.float32)
    out += inputs["bproj"].astype(np.float32)[None, None, :]
    return out, res


def kernel(**inputs):
    inputs = {k: np.asarray(v) for k, v in inputs.items()}
    out, _ = run(inputs)
    return out.astype(np.float32)
